# revision 1
# baseline (speedup 1.0000x reference)
"""Trainium2 Bass kernel for nn_Attention_66949950210549.

Dense transformer attention block:
  qkv = x @ qkv_w.T ; per-head LN on q,k ; RoPE (positions restart at N/2) ;
  softmax(q k^T * HD^-0.5 + cross-block log(0.5) bias) @ v ; proj.

Sharding: 8 cores = 2 (batch) x 4 (head groups of 4 heads).  Each core
computes its batch's qkv for its 4 heads, attention, and a partial
projection (row-parallel over the head channels); the host sums the 4
partials per batch (the proj bias is fed to exactly one core per batch).

Per-core layout strategy (fp16 attention core, fp32 accumulation):
  - all loads are SWDGE cast-on-DMA (f32 DRAM -> f16 SBUF); x / qkv_w /
    proj_w are transposed on-chip with batched PE transposes (identity
    matmul); prefix PSUM->SBUF copies go to DVE (idle there), the
    phase-D q/k transpose copies to ACT (idle there).
  - qkv matmul in natural (n, j) orientation; LN via one ACT square +
    grouped DVE tensor_reduce sums, applied on ACT as Identity with
    per-partition scale=rstd, bias=-mu*rstd; RoPE via host-precomputed
    cos/sin tables with the LN weights folded in (single shared table
    when qn and kn params match).
  - q/k re-transposed to (d, n) with head PAIRS packed into partitions
    0-63 / 64-127, so the K=64 scoresT matmuls auto-row-tile into
    concurrent PE row-groups.
  - exp on ACT straight out of PSUM with softmax scale and the
    cross-block log(0.5) bias folded into the activation (no max pass:
    LN bounds |score| <= 8, exp is overflow-safe in fp32).
  - AV: v-chunk stationary augmented with a ones column (M=65) so the
    softmax denominator accumulates in PSUM row 64 for free; normalize
    by the reciprocal rowsum (partition-broadcast via a DRAM bounce),
    pack oT (c_in, n), project with transposed proj weights, DMA out.
"""

import math
import os
import sys

sys.path.insert(0, "/opt/trn_rl_repo")

PHASES = os.environ.get("BASS_PHASES", "ABCDEF")

import numpy as np

import concourse.bacc as bacc
import concourse.bass as bass
import concourse.tile as tile
from concourse import bass_utils, mybir

B, N, C = 2, 2048, 1024
H, HD = 16, 64
NCORES = 8
GH = 4  # head-group count (cores per batch)
NH = H // GH  # heads per core = 4
J = 3 * NH * HD  # qkv rows per core = 768
NIN = N // 2  # rope positions restart here
NT = N // 128  # 16 row tiles
CCH = C // 128  # 8 contraction chunks
LOG_COND = math.log(0.5)
EPS = 1e-5
SCALE = HD ** -0.5  # 0.125

F32 = mybir.dt.float32
F16 = mybir.dt.float16
AF = mybir.ActivationFunctionType
AX = mybir.AxisListType
ALU = mybir.AluOpType


def build_nc(with_qb: bool, with_kb: bool, shared_t: bool = False, with_pb: bool = True):
    nc = bacc.Bacc("TRN2", target_bir_lowering=False, debug=False)

    x32 = nc.dram_tensor("x32", [N, C], F32, kind="ExternalInput")
    w32 = nc.dram_tensor("w32", [J, C], F32, kind="ExternalInput")
    pw32 = nc.dram_tensor("pw32", [C, NH * HD], F32, kind="ExternalInput")
    pb = nc.dram_tensor("pb", [C], F32, kind="ExternalInput")
    nkinds_q = 6 if with_qb else 4
    nkinds_k = 6 if with_kb else 4
    nrep = 2 * NH if shared_t else NH
    tq = nc.dram_tensor("tq", [NIN, nkinds_q, nrep, 32], F16, kind="ExternalInput")
    tk = None
    if not shared_t:
        tk = nc.dram_tensor("tk", [NIN, nkinds_k, NH, 32], F16, kind="ExternalInput")
    ident = nc.dram_tensor("ident", [128, 128], F16, kind="ExternalInput")
    out_p = nc.dram_tensor("out_p", [N, C], F32, kind="ExternalOutput")

    def rng(n, ph):
        return range(n if ph in PHASES else 0)

    with tile.TileContext(nc) as tc:
        with (
            tc.tile_pool(name="persist", bufs=1) as persist,
            tc.tile_pool(name="dram", bufs=1, space="DRAM") as dram,
        ):
            # ---- persistent SBUF tensors --------------------------------
            pwT_sb = persist.tile([128, 2, C], F16)  # proj_w^T (c_in, c_out)
            pb_rep = persist.tile([128, C], F32)  # bias replicated over parts
            v_sb = persist.tile([128, NT, NH, HD + 1], F16)  # v + ones col
            qT_sb = persist.tile([128, 2, N], F16)  # head-pair packed q^T
            kT_sb = persist.tile([128, 2, N], F16)
            oT_sb = persist.tile([128, 2, N], F16)  # head-pair packed o^T

            cst = persist.tile([128, 3], F32)
            nc.vector.memset(cst[:, 0:1], EPS)
            nc.vector.memset(cst[:, 1:2], 0.0)
            nc.vector.memset(cst[:, 2:3], LOG_COND)
            nc.const_aps.aps[(F32, EPS)] = cst[:, 0:1]
            nc.const_aps.aps[(F32, 0.0)] = cst[:, 1:2]
            nc.const_aps.aps[(F32, LOG_COND)] = cst[:, 2:3]

            pb_ap = pb[:]
            pb_bcast = bass.AP(
                tensor=pb_ap.tensor,
                offset=pb_ap.offset,
                ap=[[0, 128]] + list(pb_ap.ap),
            )
            nc.gpsimd.dma_start(out=pb_rep, in_=pb_bcast)
            nc.vector.memset(v_sb[:, :, :, HD : HD + 1], 1.0)
            id_sb = persist.tile([128, 128], F16)
            nc.sync.dma_start(out=id_sb, in_=ident[:, :])

            with (
                tc.tile_pool(name="wprep", bufs=6) as wprep,
                tc.tile_pool(name="mm1", bufs=1) as mm1,
            ):
                wT_sb = mm1.tile([128, CCH, J], F16)  # qkv_w^T (c, j)
                xT_sb = mm1.tile([128, CCH, N], F16)  # x^T (c, n)
                tq_sb = mm1.tile([128, NIN // 128, nkinds_q, nrep, 32], F16)
                nc.sync.dma_start(
                    out=tq_sb, in_=tq.rearrange("(t p) k h d -> p t k h d", p=128)
                )
                tk_sb = None
                if not shared_t:
                    tk_sb = mm1.tile([128, NIN // 128, nkinds_k, NH, 32], F16)
                    nc.sync.dma_start(
                        out=tk_sb, in_=tk.rearrange("(t p) k h d -> p t k h d", p=128)
                    )

                # ---- weights: cast-on-DMA load + PE transpose -----------
                with tc.tile_pool(name="tpps", bufs=3, space="PSUM") as tpps:
                    for jt in rng(J // 128, "B"):
                        wt16 = wprep.tile([128, C], F16, tag="w16t")
                        nc.gpsimd.dma_start(
                            out=wt16, in_=w32[jt * 128 : (jt + 1) * 128, :]
                        )
                        for cg in range(2):
                            tp = tpps.tile([128, 4, 128], F16, tag="tp")
                            for k in range(4):
                                ct = cg * 4 + k
                                nc.tensor.transpose(
                                    tp[:, k, :],
                                    wt16[:, ct * 128 : (ct + 1) * 128],
                                    id_sb,
                                )
                            nc.vector.tensor_copy(
                                out=wT_sb[
                                    :, cg * 4 : (cg + 1) * 4, jt * 128 : (jt + 1) * 128
                                ],
                                in_=tp,
                            )
                    for pt in rng(C // 128, "B"):
                        pwt16 = wprep.tile([128, NH * HD], F16, tag="pw16t")
                        nc.gpsimd.dma_start(
                            out=pwt16, in_=pw32[pt * 128 : (pt + 1) * 128, :]
                        )
                        tp = tpps.tile([128, 4, 128], F16, tag="tp")
                        for cc in range(2):
                            nc.tensor.transpose(
                                tp[:, cc, :],
                                pwt16[:, cc * 128 : (cc + 1) * 128],
                                id_sb,
                            )
                        nc.vector.tensor_copy(
                            out=pwT_sb[:, 0:2, pt * 128 : (pt + 1) * 128],
                            in_=tp[:, 0:2, :],
                        )

                    # ---- phase C: x load/cast + PE transpose ------------
                    for i in rng(NT, "C"):
                        x16 = wprep.tile([128, C], F16, tag="x16t")
                        nc.gpsimd.dma_start(
                            out=x16, in_=x32[i * 128 : (i + 1) * 128, :]
                        )
                        for cg in range(2):
                            tp = tpps.tile([128, 4, 128], F16, tag="tp")
                            for k in range(4):
                                ct = cg * 4 + k
                                nc.tensor.transpose(
                                    tp[:, k, :],
                                    x16[:, ct * 128 : (ct + 1) * 128],
                                    id_sb,
                                )
                            nc.vector.tensor_copy(
                                out=xT_sb[
                                    :, cg * 4 : (cg + 1) * 4, i * 128 : (i + 1) * 128
                                ],
                                in_=tp,
                            )

                # ---- phase D: qkv matmul + LN + rope ------------------------
                with (
                    tc.tile_pool(name="qkvps", bufs=3, space="PSUM") as qkvps,
                    tc.tile_pool(name="tpps2", bufs=2, space="PSUM") as tpps2,
                    tc.tile_pool(name="dwork", bufs=6) as dwork,
                ):
                    for i in rng(NT, "D"):
                        qp = qkvps.tile([128, 512], F32, tag="qp")
                        vp = qkvps.tile([128, 256], F32, tag="vp")
                        for cc in range(CCH):
                            nc.tensor.matmul(
                                qp,
                                lhsT=xT_sb[:, cc, i * 128 : (i + 1) * 128],
                                rhs=wT_sb[:, cc, 0:512],
                                start=(cc == 0),
                                stop=(cc == CCH - 1),
                            )
                            nc.tensor.matmul(
                                vp,
                                lhsT=xT_sb[:, cc, i * 128 : (i + 1) * 128],
                                rhs=wT_sb[:, cc, 512:768],
                                start=(cc == 0),
                                stop=(cc == CCH - 1),
                            )
                        # layernorm on the 8 (q,k) head groups:
                        # grouped sums on DVE, apply on ACT (scale/bias form)
                        qk_sb = dwork.tile([128, 2 * NH, HD], F16, tag="qk")
                        sq = dwork.tile([128, 2 * NH, HD], F32, tag="sq")
                        sums = dwork.tile([128, 4, 2 * NH], F32, tag="sums")
                        qp3 = qp.rearrange("p (g d) -> p g d", g=2 * NH)
                        nc.scalar.square(out=sq, in_=qp3)
                        nc.vector.tensor_reduce(
                            out=sums[:, 0, :], in_=qp3, axis=AX.X, op=ALU.add
                        )
                        nc.vector.tensor_reduce(
                            out=sums[:, 1, :], in_=sq, axis=AX.X, op=ALU.add
                        )
                        # mu = s/64 ; var = ss/64 - mu^2 ; rstd = rsqrt(var+eps)
                        nc.vector.tensor_scalar_mul(
                            out=sums[:, 0, :], in0=sums[:, 0, :], scalar1=1.0 / HD
                        )
                        nc.vector.tensor_scalar_mul(
                            out=sums[:, 1, :], in0=sums[:, 1, :], scalar1=1.0 / HD
                        )
                        nc.vector.tensor_mul(
                            out=sums[:, 2, :], in0=sums[:, 0, :], in1=sums[:, 0, :]
                        )
                        nc.vector.tensor_sub(
                            out=sums[:, 1, :], in0=sums[:, 1, :], in1=sums[:, 2, :]
                        )
                        nc.scalar.activation(
                            out=sums[:, 1, :], in_=sums[:, 1, :], func=AF.Sqrt, bias=EPS
                        )
                        nc.vector.reciprocal(out=sums[:, 1, :], in_=sums[:, 1, :])
                        # nb = -mu * rstd  (per-partition bias for the ACT apply)
                        nc.vector.tensor_mul(
                            out=sums[:, 2, :], in0=sums[:, 0, :], in1=sums[:, 1, :]
                        )
                        nc.vector.tensor_scalar_mul(
                            out=sums[:, 2, :], in0=sums[:, 2, :], scalar1=-1.0
                        )
                        for g in range(2 * NH):
                            nc.scalar.activation(
                                out=qk_sb[:, g, :],
                                in_=qp[:, g * HD : (g + 1) * HD],
                                func=AF.Identity,
                                bias=sums[:, 2, g : g + 1],
                                scale=sums[:, 1, g : g + 1],
                            )
                        # rope (tables carry the LN weights already)
                        qkr = dwork.tile([128, 2 * NH, HD], F16, tag="qkr")
                        r = i % (NIN // 128)
                        if shared_t:
                            groups = ((tq_sb, 0, 2 * NH, with_qb),)
                        else:
                            groups = (
                                (tq_sb, 0, NH, with_qb),
                                (tk_sb, NH, NH, with_kb),
                            )
                        for tsb, base, gn, wb in groups:
                            a1 = qk_sb[:, base : base + gn, 0:32]
                            a2 = qk_sb[:, base : base + gn, 32:64]
                            o1 = qkr[:, base : base + gn, 0:32]
                            o2 = qkr[:, base : base + gn, 32:64]
                            t_full = dwork.tile(
                                [128, 2 * NH, 32], F16, tag="ropetmp", name="ropetmp"
                            )
                            t = t_full[:, 0:gn, :]
                            nc.vector.tensor_mul(out=t, in0=a1, in1=tsb[:, r, 0])
                            nc.vector.tensor_mul(out=o1, in0=a2, in1=tsb[:, r, 1])
                            nc.vector.tensor_sub(out=o1, in0=t, in1=o1)
                            nc.vector.tensor_mul(out=t, in0=a2, in1=tsb[:, r, 2])
                            nc.vector.tensor_mul(out=o2, in0=a1, in1=tsb[:, r, 3])
                            nc.vector.tensor_add(out=o2, in0=t, in1=o2)
                            if wb:
                                nc.vector.tensor_add(out=o1, in0=o1, in1=tsb[:, r, 4])
                                nc.vector.tensor_add(out=o2, in0=o2, in1=tsb[:, r, 5])
                        # qT/kT via PE transpose (head pairs packed)
                        tp = tpps2.tile([128, 4, 128], F16, tag="tpqk")
                        for hp in range(2):
                            nc.tensor.transpose(
                                tp[:, hp, :],
                                qkr[:, 2 * hp : 2 * hp + 2, :],
                                id_sb,
                            )
                            nc.tensor.transpose(
                                tp[:, 2 + hp, :],
                                qkr[:, NH + 2 * hp : NH + 2 * hp + 2, :],
                                id_sb,
                            )
                        nc.scalar.copy(
                            out=qT_sb[:, 0:2, i * 128 : (i + 1) * 128],
                            in_=tp[:, 0:2, :],
                        )
                        nc.scalar.copy(
                            out=kT_sb[:, 0:2, i * 128 : (i + 1) * 128],
                            in_=tp[:, 2:4, :],
                        )
                        # v (cast to fp16, strided into the ones-augmented slots)
                        nc.vector.tensor_copy(
                            out=v_sb[:, i, :, 0:HD],
                            in_=vp.rearrange("p (h d) -> p h d", h=NH),
                        )

            # ---- phase E: attention -------------------------------------
            with (
                tc.tile_pool(name="scps", bufs=1, space="PSUM") as scps,
                tc.tile_pool(name="avps", bufs=1, space="PSUM") as avps,
                tc.tile_pool(name="epool", bufs=2) as epool,
                tc.tile_pool(name="nwork", bufs=3) as nwork,
            ):
                for nqh in rng(2, "E"):
                    for hp in range(2):
                        e_t = [
                            epool.tile([128, NT, 1024], F16, tag=f"E{z}", name=f"E{z}")
                            for z in range(2)
                        ]
                        for kc in range(NT):
                            bias = 0.0 if ((kc < 8) == (nqh == 0)) else LOG_COND
                            for z in range(2):
                                sp = scps.tile(
                                    [128, 1024], F32, tag=f"s{z}", name=f"s{z}"
                                )
                                for nqc in range(2):
                                    nq0 = nqh * 1024 + nqc * 512
                                    nc.tensor.matmul(
                                        sp[:, nqc * 512 : (nqc + 1) * 512],
                                        lhsT=kT_sb[
                                            z * 64 : (z + 1) * 64,
                                            hp,
                                            kc * 128 : (kc + 1) * 128,
                                        ],
                                        rhs=qT_sb[
                                            z * 64 : (z + 1) * 64, hp, nq0 : nq0 + 512
                                        ],
                                        start=True,
                                        stop=True,
                                    )
                                nc.scalar.activation(
                                    out=e_t[z][:, kc, :],
                                    in_=sp,
                                    func=AF.Exp,
                                    bias=bias,
                                    scale=SCALE,
                                )
                        av_t = [
                            avps.tile([128, 1024], F32, tag=f"av{z}", name=f"av{z}")
                            for z in range(2)
                        ]
                        for kc in range(NT):
                            for z in range(2):
                                for nqc in range(2):
                                    nc.tensor.matmul(
                                        av_t[z][
                                            0 : HD + 1, nqc * 512 : (nqc + 1) * 512
                                        ],
                                        lhsT=v_sb[:, kc, 2 * hp + z, :],
                                        rhs=e_t[z][
                                            :, kc, nqc * 512 : (nqc + 1) * 512
                                        ],
                                        start=(kc == 0),
                                        stop=(kc == NT - 1),
                                    )
                        # normalize: o = av[0:64] * (1/av[64]) , pack into oT_sb
                        for z in range(2):
                            rs = nwork.tile([128, 1024], F32, tag="rs")
                            nc.vector.reciprocal(
                                out=rs[HD : HD + 1, :], in_=av_t[z][HD : HD + 1, :]
                            )
                            rs_d = dram.tile([1, 1024], F32, tag="rsd", name="rs_d")
                            nc.sync.dma_start(out=rs_d, in_=rs[HD : HD + 1, :])
                            rr = nwork.tile([64, 1024], F32, tag="rr")
                            rs_ap = rs_d[:]
                            nc.gpsimd.dma_start(
                                out=rr,
                                in_=bass.AP(
                                    tensor=rs_ap.tensor,
                                    offset=rs_ap.offset,
                                    ap=[[0, 64]] + list(rs_ap.ap[1:]),
                                ),
                            )
                            o16 = nwork.tile([64, 1024], F16, tag="o16")
                            nc.vector.tensor_mul(
                                out=o16, in0=av_t[z][0:HD, :], in1=rr
                            )
                            nc.sync.dma_start(
                                out=oT_sb[
                                    z * 64 : (z + 1) * 64,
                                    hp,
                                    nqh * 1024 : (nqh + 1) * 1024,
                                ],
                                in_=o16,
                            )

            # ---- phase F: projection ------------------------------------
            with (
                tc.tile_pool(name="prps", bufs=2, space="PSUM") as prps,
                tc.tile_pool(name="fwork", bufs=4) as fwork,
            ):
                for i in rng(NT, "F"):
                    op = prps.tile([128, C], F32, tag="op")
                    for cc in range(2):
                        for oc in range(2):
                            nc.tensor.matmul(
                                op[:, oc * 512 : (oc + 1) * 512],
                                lhsT=oT_sb[:, cc, i * 128 : (i + 1) * 128],
                                rhs=pwT_sb[:, cc, oc * 512 : (oc + 1) * 512],
                                start=(cc == 0),
                                stop=(cc == 1),
                            )
                    ot = fwork.tile([128, C], F32, tag="ot")
                    if with_pb:
                        nc.vector.tensor_add(out=ot, in0=op, in1=pb_rep)
                    else:
                        nc.scalar.copy(out=ot, in_=op)
                    nc.sync.dma_start(out=out_p[i * 128 : (i + 1) * 128, :], in_=ot)

    nc.compile()
    return nc


def _rope_tables(n_w, n_b, with_b, reps=NH):
    inv = 1.0 / (10000.0 ** (np.arange(0, HD, 2, dtype=np.float64) / HD))
    ang = np.arange(NIN, dtype=np.float64)[:, None] * inv[None, :]  # (NIN, 32)
    cos_h = np.cos(ang)
    sin_h = np.sin(ang)
    w1, w2 = n_w[:32].astype(np.float64), n_w[32:].astype(np.float64)
    b1, b2 = n_b[:32].astype(np.float64), n_b[32:].astype(np.float64)
    kinds = [w1 * cos_h, w2 * sin_h, w2 * cos_h, w1 * sin_h]
    if with_b:
        kinds += [b1 * cos_h - b2 * sin_h, b2 * cos_h + b1 * sin_h]
    t = np.stack(kinds, axis=1)  # (NIN, k, 32)
    t = np.repeat(t[:, :, None, :], reps, axis=2)  # (NIN, k, reps, 32)
    return np.ascontiguousarray(t.astype(np.float16))


_NC_CACHE = {}


def kernel(x, qkv_w, qn_w, qn_b, kn_w, kn_b, proj_w, proj_b):
    x = np.asarray(x, np.float32)
    qkv_w = np.asarray(qkv_w, np.float32)
    proj_w = np.asarray(proj_w, np.float32)
    proj_b = np.asarray(proj_b, np.float32)
    qn_w = np.asarray(qn_w, np.float32)
    qn_b = np.asarray(qn_b, np.float32)
    kn_w = np.asarray(kn_w, np.float32)
    kn_b = np.asarray(kn_b, np.float32)

    with_qb = bool(np.any(qn_b != 0))
    with_kb = bool(np.any(kn_b != 0))
    shared_t = (
        with_qb == with_kb
        and np.array_equal(qn_w, kn_w)
        and np.array_equal(qn_b, kn_b)
    )
    with_pb = bool(np.any(proj_b != 0))
    key = (with_qb, with_kb, shared_t, with_pb)
    if key not in _NC_CACHE:
        _NC_CACHE[key] = build_nc(with_qb, with_kb, shared_t, with_pb)
    nc = _NC_CACHE[key]

    tq = _rope_tables(qn_w, qn_b, with_qb, reps=2 * NH if shared_t else NH)
    tk = None if shared_t else _rope_tables(kn_w, kn_b, with_kb)
    ident = np.eye(128, dtype=np.float16)

    in_maps = []
    for core in range(NCORES):
        b, g = core // GH, core % GH
        rows = slice(g * NH * HD, (g + 1) * NH * HD)
        w_core = np.ascontiguousarray(
            np.concatenate([qkv_w[rows], qkv_w[C:][rows], qkv_w[2 * C :][rows]], 0)
        )
        im = {
            "x32": np.ascontiguousarray(x[b]),
            "w32": w_core,
            "pw32": np.ascontiguousarray(proj_w[:, rows]),
            "pb": proj_b if g == 0 else np.zeros_like(proj_b),
            "tq": tq,
            "ident": ident,
        }
        if tk is not None:
            im["tk"] = tk
        in_maps.append(im)

    res = bass_utils.run_bass_kernel_spmd(nc, in_maps, core_ids=list(range(NCORES)))
    parts = [r["out_p"] for r in res.results]
    out = np.stack(
        [np.sum(parts[b * GH : (b + 1) * GH], axis=0, dtype=np.float32) for b in range(B)]
    )
    return out.astype(np.float32)


if __name__ == "__main__":
    rng = np.random.default_rng(0)
    ins = {
        "x": rng.standard_normal((B, N, C), np.float32),
        "qkv_w": (rng.standard_normal((3 * C, C), np.float32) / math.sqrt(C)).astype(
            np.float32
        ),
        "qn_w": np.ones(HD, np.float32),
        "qn_b": np.zeros(HD, np.float32),
        "kn_w": np.ones(HD, np.float32),
        "kn_b": np.zeros(HD, np.float32),
        "proj_w": (rng.standard_normal((C, C), np.float32) / math.sqrt(C)).astype(
            np.float32
        ),
        "proj_b": np.zeros(C, np.float32),
    }
    o = kernel(**ins)
    print(o.shape, o.dtype)



# revision 67
# speedup vs baseline: 1.7005x; 1.7005x over previous
"""Trainium2 Bass kernel for nn_Attention_66949950210549.

Dense transformer attention block:
  qkv = x @ qkv_w.T ; per-head LN on q,k ; RoPE (positions restart at N/2) ;
  softmax(q k^T * HD^-0.5 + cross-block log(0.5) bias) @ v ; proj.

Sharding: 8 cores = 2 (batch) x 4 (head groups of 4 heads).  Each core
computes its batch's qkv for its 4 heads, attention, and a partial
projection (row-parallel over the head channels); the host sums the 4
partials per batch (the proj bias is added on exactly one core per batch).

Design notes (cost-model driven):
  - All inputs host-cast to fp16 and host-transposed, so x^T / qkv_w^T /
    proj_w^T load straight into SBUF with no on-chip transposes.
  - qkv_w is augmented with 8 per-group row-sum columns so the LN mean
    arrives as matmul output (no extra reduction).
  - LN: square on ACT, fp16 grouped reduce on DVE; stats math batched
    over 2-tile groups (rsqrt via ACT-sqrt(scale=2^-12)+DVE-reciprocal
    giving R = 8/std); apply is per-group DVE tensor_scalar (q*R + nb,
    4x mode on SBUF fp16).
    The 8x is unfolded by w/8-scaled rope tables.  The qkv loop runs a
    software pipeline: stage A (matmul+stats inputs), batched stage B,
    stage C (apply/rope/pair-transpose) lagging A by 6 tiles, with one
    head-0 score/exp unit warming the exp pipeline during the tail.
  - Cross-block log(0.5) bias is constant within each (head, kc,
    nq-half) exp unit, so it folds into the ACT exp bias / Schraudolph
    constant (exactly -1024 int units) -- no bias matmul work at all.
  - Scores stay (nk, nq); exp split between ACT (native Exp) and DVE
    (Schraudolph: i16 = rint(s*1024*SCALE/ln2 + 15360 + tune), bitcast
    fp16 ~= exp(s*SCALE) within +-3%), ratio tuned for engine balance.
    Exp engine split is (u*13)%32 < 19 -> ACT (59%), else DVE.
  - AV out=(nq, hd+1) with a ones-column in v giving the softmax
    denominator per-partition; normalize = reciprocal + per-partition
    scale, alternating ACT/DVE.  AV of head h interleaves with scores
    of head h+1 to keep PE busy during exp.
  - o pair-transposed on PE for the row-parallel projection; output DMA
    from an SBUF bounce (copies split ACT/DVE).
"""

import math
import os
import sys

sys.path.insert(0, "/opt/trn_rl_repo")

PHASES = os.environ.get("BASS_PHASES", "LDEF")
DBG = set(os.environ.get("BASS_DBG", "").split(","))

import numpy as np

import concourse.bacc as bacc
import concourse.bass as bass
import concourse.tile as tile
from concourse import bass_utils, mybir

B, N, C = 2, 2048, 1024
H, HD = 16, 64
NCORES = 8
GH = 4  # head-group count (cores per batch)
NH = H // GH  # heads per core = 4
J = 3 * NH * HD + 8  # qkv rows per core + 8 LN-mean columns = 776
NIN = N // 2  # rope positions restart here
NT = N // 128  # 16 row tiles
CCH = C // 128  # 8 contraction chunks
LOG_COND = math.log(0.5)
EPS = 1e-5
SCALE = HD ** -0.5  # 0.125
NG = 2 * NH  # 8 LN groups (4 q heads + 4 k heads)
KA = HD + 2  # 66: head dim + 2 bias-aug dims

# Schraudolph exp: bitcast(int16(s*C1 + C2)) ~= exp(s*SCALE)
SCH_C1 = 1024.0 * SCALE / math.log(2.0)
SCH_C2 = 15360.0 - 44.0
# exp engine assignment over units: ACT when (u*5) % ACT_MOD < ACT_THR
ACT_MOD = 32
ACT_THR = int(os.environ.get("BASS_ACT_THR", "19"))

F32 = mybir.dt.float32
F16 = mybir.dt.float16
I16 = mybir.dt.int16
AF = mybir.ActivationFunctionType
AX = mybir.AxisListType
ALU = mybir.AluOpType


def build_nc(with_qb: bool, with_kb: bool, shared_t: bool = False, with_pb: bool = True):
    nc = bacc.Bacc("TRN2", target_bir_lowering=False, debug=False)

    xT = nc.dram_tensor("xT", [NT, 128, C], F16, kind="ExternalInput")  # tiled x^T
    wT = nc.dram_tensor("wT", [C, J], F16, kind="ExternalInput")  # qkv_w^T + sums
    pwT = nc.dram_tensor("pwT", [NH * HD, C], F16, kind="ExternalInput")
    nkinds_q = 6 if with_qb else 4
    nkinds_k = 6 if with_kb else 4
    nrep = NG if shared_t else NH
    tq = nc.dram_tensor("tq", [NIN, nkinds_q, nrep, 32], F16, kind="ExternalInput")
    tk = None
    if not shared_t:
        tk = nc.dram_tensor("tk", [NIN, nkinds_k, nrep, 32], F16, kind="ExternalInput")
    ident = nc.dram_tensor("ident", [128, 128], F16, kind="ExternalInput")
    pb = None
    if with_pb:
        pb = nc.dram_tensor("pb", [C], F32, kind="ExternalInput")
    out_p = nc.dram_tensor("out_p", [N, C], F16, kind="ExternalOutput")

    def rng(n, ph):
        return range(n if ph in PHASES else 0)

    ctx = nc.allow_low_precision("fp16 LN stats / softmax weights are accurate enough")
    ctx.__enter__()
    with tile.TileContext(nc) as tc:
        with (
            tc.tile_pool(name="persist", bufs=1) as persist,
        ):
            # ---- persistent SBUF tensors --------------------------------
            pwT_sb = persist.tile([128, 2, C], F16)  # proj_w^T (c_in, c_out)
            v_sb = persist.tile([128, NT, NH, HD + 1], F16)  # v + ones col
            qkT_sb = persist.tile([128, NT, NG // 2, 128], F16)  # pair-packed q/k^T
            o_sb = persist.tile([128, NT, NH, HD], F16)  # normalized o (n, h, hd)
            oT_sb = persist.tile([128, 2, N], F16)  # head-pair packed o^T
            id_sb = persist.tile([128, 128], F16)

            cst = persist.tile([128, 4], F32)
            # touch GPSIMD with a real compute op first so the library load
            # (and its ucode DMA) runs before the big input DMAs
            lw = persist.tile([128, 1], F32)
            nc.gpsimd.memset(lw, 0.0)
            nc.gpsimd.tensor_scalar(
                out=lw, in0=lw, scalar1=0.0, scalar2=None, op0=ALU.mult
            )
            nc.vector.memset(cst[:, 0:1], 0.0)
            nc.vector.memset(cst[:, 1:2], 1.0)
            nc.vector.memset(cst[:, 2:3], EPS / HD)
            nc.vector.memset(cst[:, 3:4], LOG_COND)
            nc.const_aps.aps[(F32, 0.0)] = cst[:, 0:1]
            nc.const_aps.aps[(F32, 1.0)] = cst[:, 1:2]
            nc.const_aps.aps[(F32, EPS / HD)] = cst[:, 2:3]
            nc.const_aps.aps[(F32, LOG_COND)] = cst[:, 3:4]

            nc.sync.dma_start(out=id_sb, in_=ident[:, :])
            nc.vector.memset(v_sb[:, :, :, HD : HD + 1], 1.0)

            pb_rep = None
            if with_pb:
                pb_ap = pb[:]
                pb_rep = persist.tile([128, C], F32)
                nc.gpsimd.dma_start(
                    out=pb_rep,
                    in_=bass.AP(
                        tensor=pb_ap.tensor,
                        offset=pb_ap.offset,
                        ap=[[0, 128]] + list(pb_ap.ap),
                    ),
                )

            # ---- exp unit emission (shared by D-overlap and E) ----------
            ucnt = [0, 0]
            e_t = {}  # (h, z) -> half-head exp'd score tile [128, NT, 1024]

            def e1_unit(hz, kc, scpool, force_dve=False):
                h, z = hz
                e_h = e_t[hz]
                cross = (kc < NT // 2) != (z == 0)
                kg, qg = NH + h, h
                sc = scpool.tile([128, 1024], F32, tag="sc")
                for w2 in range(2):
                    t0 = z * 8 + w2 * 4
                    nc.tensor.matmul(
                        sc[:, w2 * 512 : (w2 + 1) * 512],
                        lhsT=qkT_sb[
                            (kg % 2) * HD : (kg % 2 + 1) * HD, kc, kg // 2, :
                        ],
                        rhs=qkT_sb[
                            (qg % 2) * HD : (qg % 2 + 1) * HD, t0 : t0 + 4, qg // 2, :
                        ],
                        start=True,
                        stop=True,
                    )
                u = ucnt[0]
                ucnt[0] += 1
                if not force_dve and (u * 13) % ACT_MOD < ACT_THR:
                    nc.scalar.activation(
                        out=e_h[:, kc, :],
                        in_=sc,
                        func=AF.Exp,
                        scale=SCALE,
                        bias=LOG_COND if cross else 0.0,
                    )
                else:
                    nc.vector.tensor_scalar(
                        out=e_h.bitcast(I16)[:, kc, :],
                        in0=sc,
                        scalar1=SCH_C1,
                        scalar2=SCH_C2 - (1024.0 if cross else 0.0),
                        op0=ALU.mult,
                        op1=ALU.add,
                    )

            epool_cm = tc.tile_pool(name="epool", bufs=2)
            epool = epool_cm.__enter__()

            def new_e(hz):
                e_t[hz] = epool.tile(
                    [128, NT, 1024], F16, tag="e", name=f"e{(2 * hz[0] + hz[1]) % 2}"
                )

            with tc.tile_pool(name="mm1", bufs=1) as mm1:
                wT_sb = mm1.tile([128, CCH, J], F16)  # qkv_w^T (c, j)
                xT_sb = mm1.tile([128, NT, CCH, 128], F16)  # x^T per n-tile
                tq_sb = mm1.tile([128, NIN // 128, nkinds_q, nrep, 32], F16)
                tk_sb = None

                if "L" in PHASES:
                    # first n-tile of x + weights first so D starts early;
                    # rope tables after 4 x-tiles (stage C needs them ~12us in)
                    nc.sync.dma_start(
                        out=xT_sb[:, 0, :, :],
                        in_=xT[0].rearrange("p (cc n) -> p cc n", cc=CCH),
                    )
                    for cc in range(CCH):
                        nc.sync.dma_start(
                            out=wT_sb[:, cc, :],
                            in_=wT[cc * 128 : (cc + 1) * 128, :],
                        )
                    for i in range(1, 4):
                        nc.sync.dma_start(
                            out=xT_sb[:, i, :, :],
                            in_=xT[i].rearrange("p (cc n) -> p cc n", cc=CCH),
                        )
                    nc.sync.dma_start(
                        out=tq_sb, in_=tq.rearrange("(t p) k h d -> p t k h d", p=128)
                    )
                    if not shared_t:
                        tk_sb = mm1.tile([128, NIN // 128, nkinds_k, nrep, 32], F16)
                        nc.sync.dma_start(
                            out=tk_sb,
                            in_=tk.rearrange("(t p) k h d -> p t k h d", p=128),
                        )
                    for i in range(4, NT):
                        nc.sync.dma_start(
                            out=xT_sb[:, i, :, :],
                            in_=xT[i].rearrange("p (cc n) -> p cc n", cc=CCH),
                        )
                    nc.sync.dma_start(
                        out=pwT_sb,
                        in_=pwT.rearrange("(cc p) j -> p cc j", p=128),
                    )

                # ---- phase D: qkv matmul + LN + rope --------------------
                with (
                    tc.tile_pool(name="qkvps", bufs=2, space="PSUM") as qkvps,
                    tc.tile_pool(name="escps", bufs=1, space="PSUM") as escps,
                    tc.tile_pool(name="tpps", bufs=2, space="PSUM") as tpps,
                    tc.tile_pool(name="dwork", bufs=3) as dwork,
                    tc.tile_pool(name="q16p", bufs=8) as q16p,
                ):
                    TG = 4  # stats batch group size
                    stb_all = mm1.tile([128, 6, NT, NG], F32)  # batched LN stats
                    q16_t = {}

                    def d_stageA(i):
                        qp = qkvps.tile([128, 512], F32, tag="qp")
                        vp = qkvps.tile([128, 264], F32, tag="vp")
                        for cc in range(CCH):
                            nc.tensor.matmul(
                                qp,
                                lhsT=xT_sb[:, i, cc, :],
                                rhs=wT_sb[:, cc, 0:512],
                                start=(cc == 0),
                                stop=(cc == CCH - 1),
                            )
                            nc.tensor.matmul(
                                vp,
                                lhsT=xT_sb[:, i, cc, :],
                                rhs=wT_sb[:, cc, 512:776],
                                start=(cc == 0),
                                stop=(cc == CCH - 1),
                            )
                        sq = dwork.tile([128, NG, HD], F16, tag="sq")
                        qp3 = qp.rearrange("p (g d) -> p g d", g=NG)
                        nc.scalar.activation(
                            out=sq, in_=qp3, func=AF.Square, scale=8.0
                        )  # (8q)^2 -> reduce gives 64*ss
                        nc.vector.tensor_reduce(
                            out=stb_all[:, 0, i, :], in_=sq, axis=AX.X, op=ALU.add
                        )  # ss
                        nc.vector.tensor_copy(
                            out=stb_all[:, 1, i, :], in_=vp[:, 256:264]
                        )  # s
                        q16 = q16p.tile([128, NG, HD], F16, tag="q16")
                        nc.scalar.copy(out=q16, in_=qp3)
                        q16_t[i] = q16
                        nc.scalar.copy(
                            out=v_sb[:, i, :, 0:HD],
                            in_=vp[:, 0:256].rearrange("p (h d) -> p h d", h=NH),
                        )

                    def d_stageB(g):
                        sl = slice(g * TG, (g + 1) * TG)
                        ss = stb_all[:, 0, sl, :]
                        s = stb_all[:, 1, sl, :]
                        var = stb_all[:, 2, sl, :]
                        R = stb_all[:, 3, sl, :]
                        nb = stb_all[:, 4, sl, :]
                        if "NOSTATS" in DBG:
                            nc.vector.memset(stb_all[:, 2:5, sl, :], 1.0)
                            return
                        nc.vector.tensor_mul(out=var, in0=s, in1=s)  # s^2
                        nc.vector.tensor_sub(out=var, in0=ss, in1=var)  # 64*var64
                        nc.scalar.activation(
                            out=R, in_=var, func=AF.Sqrt,
                            bias=EPS / HD, scale=1.0 / 4096.0 / HD,
                        )
                        nc.vector.reciprocal(out=R, in_=R)  # R = 8/std
                        nc.vector.tensor_mul(out=nb, in0=s, in1=R)
                        nc.vector.tensor_scalar(
                            out=nb, in0=nb, scalar1=-1.0 / HD, scalar2=None,
                            op0=ALU.mult,
                        )  # nb = -mu*R

                    def d_stageC(i):
                        q16 = q16_t.pop(i)
                        qn = dwork.tile([128, NG, HD], F16, tag="qn")
                        if "NOAPPLY" in DBG:
                            nc.vector.tensor_copy(out=qn, in_=q16)
                        else:
                            for g in range(NG):
                                nc.vector.tensor_scalar(
                                    out=qn[:, g, :],
                                    in0=q16[:, g, :],
                                    scalar1=stb_all[:, 3, i, g : g + 1],
                                    scalar2=stb_all[:, 4, i, g : g + 1],
                                    op0=ALU.mult,
                                    op1=ALU.add,
                                )
                        qkr = dwork.tile([128, NG, HD], F16, tag="qkr")
                        r = i % (NIN // 128)
                        if shared_t:
                            groups = ((tq_sb, 0, NG, with_qb),)
                        else:
                            groups = (
                                (tq_sb, 0, NH, with_qb),
                                (tk_sb, NH, NH, with_kb),
                            )
                        t_full = dwork.tile([128, NG, 32], F16, tag="ropetmp")
                        t2_full = dwork.tile([128, NG, 32], F16, tag="ropetmp2")
                        if "NOROPE" in DBG:
                            nc.vector.tensor_copy(out=qkr[:, :, 0:64], in_=qn)
                            groups = ()
                        for tsb, base, gn, wb in groups:
                            a1 = qn[:, base : base + gn, 0:32]
                            a2 = qn[:, base : base + gn, 32:64]
                            o1 = qkr[:, base : base + gn, 0:32]
                            o2 = qkr[:, base : base + gn, 32:64]
                            t = t_full[:, base : base + gn, :]
                            t2 = t2_full[:, base : base + gn, :]
                            nc.gpsimd.tensor_mul(out=t, in0=a1, in1=tsb[:, r, 0])
                            nc.vector.tensor_mul(out=o1, in0=a2, in1=tsb[:, r, 1])
                            nc.vector.tensor_sub(out=o1, in0=t, in1=o1)
                            nc.gpsimd.tensor_mul(out=t2, in0=a2, in1=tsb[:, r, 2])
                            nc.vector.tensor_mul(out=o2, in0=a1, in1=tsb[:, r, 3])
                            nc.vector.tensor_add(out=o2, in0=t2, in1=o2)
                            if wb:
                                nc.vector.tensor_add(out=o1, in0=o1, in1=tsb[:, r, 4])
                                nc.vector.tensor_add(out=o2, in0=o2, in1=tsb[:, r, 5])
                        tp = tpps.tile([128, NG // 2, 128], F16, tag="tpqk")
                        for p in range(NG // 2):
                            nc.tensor.transpose(
                                tp[:, p, :], qkr[:, 2 * p : 2 * p + 2, :], id_sb
                            )
                        nc.scalar.copy(
                            out=qkT_sb[:, i, :, :], in_=tp
                        )

                    if "D" in PHASES:
                        NGP = NT // TG  # 4 groups
                        LAG = TG + 1
                        for g in range(NGP):
                            for i in range(g * TG, (g + 1) * TG):
                                d_stageA(i)
                                if i - LAG >= 0:
                                    d_stageC(i - LAG)
                            d_stageB(g)
                        ek = 0
                        for ci in range(NT - LAG, NT):
                            d_stageC(ci)
                            # warm the exp pipeline with head-0 score units
                            if "E" in PHASES and ci >= int(os.environ.get("BASS_ECI", "15")):
                                if ek == 0:
                                    new_e((0, 0))
                                e1_unit((0, 0), ek, escps)
                                ek += 1
                        ucnt[1] = ek

            # ---- phase E: attention (mm1 freed) -------------------------
            with (
                tc.tile_pool(name="scps", bufs=3, space="PSUM") as scps,
                tc.tile_pool(name="avps", bufs=2, space="PSUM") as avps,
                tc.tile_pool(name="nwork", bufs=4) as nwork,
            ):
                def e2_tile(hz, t):
                    h, z = hz
                    e_h = e_t[hz]
                    tt = t - z * 8
                    av = avps.tile([128, HD + 1], F32, tag="av")
                    for kc in range(NT):
                        nc.tensor.matmul(
                            av,
                            lhsT=e_h[:, kc, tt * 128 : (tt + 1) * 128],
                            rhs=v_sb[:, kc, h, :],
                            start=(kc == 0),
                            stop=(kc == NT - 1),
                        )
                    rr = nwork.tile([128, 1], F32, tag="rr")
                    nc.vector.reciprocal(out=rr, in_=av[:, HD : HD + 1])
                    if t % 3 == 0:
                        nc.scalar.activation(
                            out=o_sb[:, t, h, :], in_=av[:, 0:HD],
                            func=AF.Identity, scale=rr,
                        )
                    else:
                        nc.vector.tensor_scalar(
                            out=o_sb[:, t, h, :], in0=av[:, 0:HD],
                            scalar1=rr, scalar2=None, op0=ALU.mult,
                        )

                if "E" in PHASES:
                    units = [(h, z) for h in range(NH) for z in range(2)]
                    if ucnt[1] == 0:
                        new_e((0, 0))
                    for kc in range(ucnt[1], NT):
                        e1_unit((0, 0), kc, scps)
                    # pipeline: scores of unit ui interleaved with AV of ui-1
                    for ui in range(1, len(units)):
                        cur, prev = units[ui], units[ui - 1]
                        new_e(cur)
                        for step in range(NT):
                            e1_unit(cur, step, scps)
                            if step % 2 == 1:
                                e2_tile(prev, prev[1] * 8 + step // 2)
                        e_t.pop(prev)
                    last = units[-1]
                    for t in range(8):
                        e2_tile(last, last[1] * 8 + t)
                    e_t.pop(last)
            epool_cm.__exit__(None, None, None)

            # ---- phase F: o^T transpose + projection --------------------
            with (
                tc.tile_pool(name="prps", bufs=2, space="PSUM") as prps,
                tc.tile_pool(name="tp2ps", bufs=2, space="PSUM") as tp2ps,
                tc.tile_pool(name="fwork", bufs=int(os.environ.get("BASS_FWB","3"))) as fwork,
            ):
                for i in rng(NT, "F"):
                    tp = tp2ps.tile([128, 2, 128], F16, tag="tpo")
                    for p in range(2):
                        nc.tensor.transpose(
                            tp[:, p, :],
                            o_sb[:, i, 2 * p : 2 * p + 2, :],
                            id_sb,
                        )
                    nc.vector.tensor_copy(
                        out=oT_sb[:, :, i * 128 : (i + 1) * 128],
                        in_=tp,
                    )
                for i in rng(NT, "F"):
                    op = prps.tile([128, C], F32, tag="op")
                    for cc in range(2):
                        for oc in range(2):
                            nc.tensor.matmul(
                                op[:, oc * 512 : (oc + 1) * 512],
                                lhsT=oT_sb[:, cc, i * 128 : (i + 1) * 128],
                                rhs=pwT_sb[:, cc, oc * 512 : (oc + 1) * 512],
                                start=(cc == 0),
                                stop=(cc == 1),
                            )
                    ot = fwork.tile([128, C], F16, tag="ot")
                    if with_pb:
                        nc.vector.tensor_add(out=ot, in0=op, in1=pb_rep)
                    elif i % 2 == 0:
                        nc.scalar.copy(out=ot, in_=op)
                    else:
                        nc.vector.tensor_copy(out=ot, in_=op)
                    nc.sync.dma_start(out=out_p[i * 128 : (i + 1) * 128, :], in_=ot)

    ctx.__exit__(None, None, None)
    nc.compile()
    return nc


def _rope_tables(n_w, n_b, with_b, reps):
    inv = 1.0 / (10000.0 ** (np.arange(0, HD, 2, dtype=np.float64) / HD))
    ang = np.arange(NIN, dtype=np.float64)[:, None] * inv[None, :]  # (NIN, 32)
    cos_h = np.cos(ang)
    sin_h = np.sin(ang)
    # w/8 scaling unfolds the 8x from R = 8/std
    w1 = n_w[:32].astype(np.float64) / 8.0
    w2 = n_w[32:].astype(np.float64) / 8.0
    b1, b2 = n_b[:32].astype(np.float64), n_b[32:].astype(np.float64)
    kinds = [w1 * cos_h, w2 * sin_h, w2 * cos_h, w1 * sin_h]
    if with_b:
        kinds += [b1 * cos_h - b2 * sin_h, b2 * cos_h + b1 * sin_h]
    t = np.stack(kinds, axis=1)  # (NIN, k, 32)
    t = np.repeat(t[:, :, None, :], reps, axis=2)  # (NIN, k, reps, 32)
    return np.ascontiguousarray(t.astype(np.float16))


_NC_CACHE = {}


def kernel(x, qkv_w, qn_w, qn_b, kn_w, kn_b, proj_w, proj_b):
    x = np.asarray(x, np.float32)
    qkv_w = np.asarray(qkv_w, np.float32)
    proj_w = np.asarray(proj_w, np.float32)
    proj_b = np.asarray(proj_b, np.float32)
    qn_w = np.asarray(qn_w, np.float32)
    qn_b = np.asarray(qn_b, np.float32)
    kn_w = np.asarray(kn_w, np.float32)
    kn_b = np.asarray(kn_b, np.float32)

    with_qb = bool(np.any(qn_b != 0))
    with_kb = bool(np.any(kn_b != 0))
    shared_t = (
        with_qb == with_kb
        and np.array_equal(qn_w, kn_w)
        and np.array_equal(qn_b, kn_b)
    )
    with_pb = bool(np.any(proj_b != 0))
    key = (with_qb, with_kb, shared_t, with_pb)
    if key not in _NC_CACHE:
        _NC_CACHE[key] = build_nc(with_qb, with_kb, shared_t, with_pb)
    nc = _NC_CACHE[key]

    nrep = NG if shared_t else NH
    tq = _rope_tables(qn_w, qn_b, with_qb, reps=nrep)
    tk = None if shared_t else _rope_tables(kn_w, kn_b, with_kb, reps=nrep)
    ident = np.eye(128, dtype=np.float16)

    in_maps = []
    for core in range(NCORES):
        b, g = core // GH, core % GH
        rows = slice(g * NH * HD, (g + 1) * NH * HD)
        w_core = np.concatenate(
            [qkv_w[rows], qkv_w[C:][rows], qkv_w[2 * C :][rows]], 0
        )  # (768, C) rows = [q(256), k(256), v(256)]
        # 8 LN-mean columns: row sums per 64-wide group of q then k
        sums = w_core[0:512].reshape(NG, HD, C).sum(axis=1)  # (8, C)
        w_aug = np.concatenate([w_core, sums], 0)  # (776, C)
        # x^T packed so each n-tile is one contiguous 2KB-per-partition DMA:
        # [nt, p(c within chunk), cc*128(n)]
        xt = x[b].T.astype(np.float16).reshape(CCH, 128, NT, 128)
        xt = np.ascontiguousarray(xt.transpose(2, 1, 0, 3).reshape(NT, 128, C))
        im = {
            "xT": xt,
            "wT": np.ascontiguousarray(w_aug.T.astype(np.float16)),
            "pwT": np.ascontiguousarray(proj_w[:, rows].T.astype(np.float16)),
            "tq": tq,
            "ident": ident,
        }
        if with_pb:
            im["pb"] = proj_b if g == 0 else np.zeros_like(proj_b)
        if tk is not None:
            im["tk"] = tk
        in_maps.append(im)

    res = bass_utils.run_bass_kernel_spmd(nc, in_maps, core_ids=list(range(NCORES)))
    parts = [r["out_p"].astype(np.float32) for r in res.results]
    out = np.stack(
        [np.sum(parts[b * GH : (b + 1) * GH], axis=0, dtype=np.float32) for b in range(B)]
    )
    return out.astype(np.float32)


if __name__ == "__main__":
    rng = np.random.default_rng(0)
    ins = {
        "x": rng.standard_normal((B, N, C), np.float32),
        "qkv_w": (rng.standard_normal((3 * C, C), np.float32) / math.sqrt(C)).astype(
            np.float32
        ),
        "qn_w": np.ones(HD, np.float32),
        "qn_b": np.zeros(HD, np.float32),
        "kn_w": np.ones(HD, np.float32),
        "kn_b": np.zeros(HD, np.float32),
        "proj_w": (rng.standard_normal((C, C), np.float32) / math.sqrt(C)).astype(
            np.float32
        ),
        "proj_b": np.zeros(C, np.float32),
    }
    o = kernel(**ins)
    print(o.shape, o.dtype)
